# revision 14
# baseline (speedup 1.0000x reference)
"""Differentiable voxel rasterizer — Trainium2 Bass kernel (8 NeuronCores).

Contract: kernel(**inputs) takes FULL inputs (positions [512,3], sizes [512],
densities [512], colors [512,3], camera_matrix [4,4], intrinsics [3,3]) and
returns the FULL output tuple (rgb [256,256,3], depth [256,256],
alpha [256,256]) matching reference.reference().

Strategy (sharding = image plane): each of 8 cores renders a 32-row band.
Within a band, 64 tiles of 8x16 px (partition dim = 128 pixels). Host
projects voxels, culls per tile (exact: outside the circular footprint the
reference weight is identically 0), sorts back-to-front, and packs per-tile
tables. Device computes, per tile:
    d2|t       via one PE matmul (rank-4 expansion, tile-centered coords)
    dist       = sqrt(relu(d2))            (ACT)
    pen        = relu(BIG*t)               (DVE, t = d2-hs^2 mask penalty)
    E          = exp(-0.5*(dist+pen))      (ACT; gpsimd add)
    W          = E*va                      (gpsimd)
    P          = prefix prod of (1-W)      (DVE tensor_tensor_scan)
    blend_v    = P_{v-1}-P_v (telescoping) -> omb = 1-blend via one stt
    R          = suffix prod of omb        (DVE reversed scan)
    BQ_v       = R_{v+1}-R_v (telescoping) = blend_v * prod_{u>v} omb_u
    rgb/depth  = matmul(BQ^T, [c | d-FAR]) (PE transpose + PE matmul)
    alpha      = 1 - P_total
Everything is NaN-free by construction (device NaNs crash the NRT).
"""
import os
import sys

for _p in ("/opt/trn_rl_repo", os.path.expanduser("~/.axon_site/_ro/trn_rl_repo")):
    if os.path.isdir(_p) and _p not in sys.path:
        sys.path.insert(0, _p)

import numpy as np

H, W_IMG = 256, 256
NEAR, FAR = 0.1, 100.0
NVOX = 512
NCORES = 8
BAND = H // NCORES          # 32 rows per core
TR, TC = 4, 16              # tile grid per band
R_, C_ = 8, 16              # tile pixel shape (R_*C_ = 128 partitions)
T = TR * TC                 # 64 tiles per core
P = 128
BIG = 1e8

_nc_cache: dict = {}


def _build(V):
    """Build + compile the per-core module for V voxels per tile."""
    import concourse.bacc as bacc
    import concourse.bass as bass
    import concourse.mybir as mybir
    from concourse.tile import TileContext

    F32 = mybir.dt.float32
    Alu = mybir.AluOpType
    Act = mybir.ActivationFunctionType
    VC = (V + P - 1) // P          # voxel chunks for transpose/reduction
    assert V <= 512

    nc = bacc.Bacc("TRN2", target_bir_lowering=False, debug=False)
    # pxf is shared by all tiles: coords are tile-centered, so the pixel
    # feature table is identical for every tile/core.
    pxf_d = nc.dram_tensor("pxf", [4, P], F32, kind="ExternalInput")
    rhs_d = nc.dram_tensor("rhs8", [T, 4, 2 * V], F32, kind="ExternalInput")
    vab_d = nc.dram_tensor("vab", [T, V], F32, kind="ExternalInput")
    vals_d = nc.dram_tensor("vals", [T, VC * P if V > P else V, 4], F32,
                            kind="ExternalInput")
    id_d = nc.dram_tensor("ident", [P, P], F32, kind="ExternalInput")
    # outputs in device-natural layout [pixel-in-tile, tile(,ch)];
    # host unscatters to image layout during gather
    rgb_d = nc.dram_tensor("rgb", [P, T, 3], F32, kind="ExternalOutput")
    dep_d = nc.dram_tensor("dep", [P, T], F32, kind="ExternalOutput")
    alp_d = nc.dram_tensor("alp", [P, T], F32, kind="ExternalOutput")

    with TileContext(nc) as tc:
        with tc.tile_pool(name="const", bufs=1) as cpool, \
             tc.tile_pool(name="work", bufs=3) as pool, \
             tc.tile_pool(name="pwork", bufs=2, space="PSUM") as ppool, \
             tc.tile_pool(name="pout", bufs=1, space="PSUM") as opool:
            ident = cpool.tile([P, P], F32)
            zeros = cpool.tile([P, V], F32)
            astage = cpool.tile([P, T], F32)
            pxf = cpool.tile([4, P], F32)
            nc.sync.dma_start(out=ident[:], in_=id_d.ap())
            nc.sync.dma_start(out=pxf[:], in_=pxf_d.ap())
            nc.vector.memset(zeros[:], 0)
            outp = opool.tile([P, 4 * T], F32)

            for t in range(T):
                rhs8 = pool.tile([4, 2 * V], F32)
                vab = pool.tile([P, V], F32)
                vals = pool.tile([P, VC, 4], F32)
                nc.sync.dma_start(out=rhs8[:], in_=rhs_d.ap()[t])
                nc.sync.dma_start(
                    out=vals[0:min(P, V), :, :],
                    in_=vals_d.ap()[t].rearrange("(c vp) f -> vp c f", c=VC))
                row = vab_d.ap()[t:t + 1, :]
                nc.sync.dma_start(
                    out=vab[:],
                    in_=bass.AP(row.tensor, row.offset, [[0, P], [1, V]]))

                d2t = ppool.tile([P, 2 * V], F32)
                nc.tensor.matmul(d2t[:], lhsT=pxf[:], rhs=rhs8[:],
                                 start=True, stop=True)

                d2c = pool.tile([P, V], F32)
                dist = pool.tile([P, V], F32)
                pen = pool.tile([P, V], F32)
                de = pool.tile([P, V], F32)
                E = pool.tile([P, V], F32)
                Wt = pool.tile([P, V], F32)
                omw = pool.tile([P, V], F32)
                Pbuf = pool.tile([P, V + 1], F32)
                omb = pool.tile([P, V], F32)
                Qbuf = pool.tile([P, V + 1], F32)
                BQ = pool.tile([P, V], F32)

                nc.scalar.activation(out=d2c[:], in_=d2t[:, 0:V], func=Act.Relu)
                nc.scalar.activation(out=dist[:], in_=d2c[:], func=Act.Sqrt)
                nc.vector.tensor_scalar(out=pen[:], in0=d2t[:, V:2 * V],
                                        scalar1=BIG, scalar2=0.0,
                                        op0=Alu.mult, op1=Alu.max)
                nc.gpsimd.tensor_tensor(out=de[:], in0=dist[:], in1=pen[:],
                                        op=Alu.add)
                nc.scalar.activation(out=E[:], in_=de[:], func=Act.Exp,
                                     scale=-0.5)
                nc.gpsimd.tensor_tensor(out=Wt[:], in0=E[:], in1=vab[:],
                                        op=Alu.mult)
                nc.vector.tensor_scalar(out=omw[:], in0=Wt[:], scalar1=-1.0,
                                        scalar2=1.0, op0=Alu.mult, op1=Alu.add)
                nc.gpsimd.memset(Pbuf[:, 0:1], 1.0)
                nc.gpsimd.memset(Qbuf[:, V:V + 1], 1.0)
                nc.vector.tensor_tensor_scan(
                    out=Pbuf[:, 1:V + 1], data0=omw[:], data1=zeros[:],
                    initial=1.0, op0=Alu.mult, op1=Alu.max)
                nc.vector.scalar_tensor_tensor(
                    out=omb[:], in0=Pbuf[:, 1:V + 1], scalar=1.0,
                    in1=Pbuf[:, 0:V], op0=Alu.add, op1=Alu.subtract)
                nc.vector.tensor_tensor_scan(
                    out=Qbuf[:, 0:V][:, ::-1], data0=omb[:][:, ::-1],
                    data1=zeros[:], initial=1.0, op0=Alu.mult, op1=Alu.max)
                nc.vector.tensor_tensor(out=BQ[:], in0=Qbuf[:, 1:V + 1],
                                        in1=Qbuf[:, 0:V], op=Alu.subtract)
                nc.gpsimd.tensor_copy(out=astage[:, t:t + 1],
                                      in_=Pbuf[:, V:V + 1])

                bqT = ppool.tile([P, VC * P], F32)
                bqTs = pool.tile([P, VC * P], F32)
                for ch in range(VC):
                    cl = min(P, V - ch * P)
                    nc.tensor.transpose(bqT[0:cl, ch * P:ch * P + P],
                                        BQ[:, ch * P:ch * P + cl], ident[:])
                    nc.scalar.copy(out=bqTs[0:cl, ch * P:ch * P + P],
                                   in_=bqT[0:cl, ch * P:ch * P + P])
                for ch in range(VC):
                    cl = min(P, V - ch * P)
                    nc.tensor.matmul(outp[:, 4 * t:4 * t + 4],
                                     lhsT=bqTs[0:cl, ch * P:ch * P + P],
                                     rhs=vals[0:cl, ch, :],
                                     start=(ch == 0), stop=(ch == VC - 1))

            rgbsb = cpool.tile([P, T, 3], F32)
            depsb = cpool.tile([P, T], F32)
            alpsb = cpool.tile([P, T], F32)
            outv = outp[:].rearrange("p (t f) -> p t f", f=4)
            nc.vector.tensor_copy(out=rgbsb[:], in_=outv[:, :, 0:3])
            nc.vector.tensor_scalar(out=depsb[:], in0=outv[:, :, 3:4],
                                    scalar1=1.0, scalar2=FAR,
                                    op0=Alu.mult, op1=Alu.add)
            nc.vector.tensor_scalar(out=alpsb[:], in0=astage[:], scalar1=-1.0,
                                    scalar2=1.0, op0=Alu.mult, op1=Alu.add)
            nc.sync.dma_start(out=rgb_d.ap(), in_=rgbsb[:])
            nc.sync.dma_start(out=dep_d.ap(), in_=depsb[:])
            nc.sync.dma_start(out=alp_d.ap(), in_=alpsb[:])
    nc.compile()
    return nc


def _softplus(x):
    return np.logaddexp(0.0, x)


def _prep(positions, sizes, densities, colors, camera_matrix, intrinsics):
    """Mirror the reference projection in numpy fp32; build per-tile tables."""
    f32 = np.float32
    pos = positions.astype(f32)
    n = pos.shape[0]
    hom = np.concatenate([pos, np.ones((n, 1), f32)], axis=1)
    cam = hom @ camera_matrix.astype(f32).T
    with np.errstate(divide="ignore", invalid="ignore", over="ignore"):
        cam3 = cam[:, :3] / cam[:, 3:4]
        scr = cam3 @ intrinsics.astype(f32).T
        sp = scr[:, :2] / scr[:, 2:3]
        depths = cam3[:, 2]
        fx = intrinsics.astype(f32)[0, 0]
        ssize = sizes.astype(f32) * fx / np.maximum(depths, f32(0.1))
        x, y = sp[:, 0], sp[:, 1]
        vis = ((depths > NEAR) & (depths < FAR)
               & (x + ssize >= 0) & (x - ssize < W_IMG)
               & (y + ssize >= 0) & (y - ssize < H))
        vis = vis & np.isfinite(x) & np.isfinite(y) & np.isfinite(ssize)
        valpha = np.clip(
            1.0 - np.exp(-_softplus(densities.astype(f32)) * sizes.astype(f32)),
            0.0, 1.0) * vis.astype(f32)
    order = np.argsort(-np.where(np.isnan(depths), -np.inf, depths),
                       kind="stable")
    sp_s = sp[order].astype(np.float64)
    hs_s = 0.5 * ssize[order].astype(np.float64)
    va_s = valpha[order].astype(np.float64)
    d_s = depths[order].astype(np.float64)
    c_s = colors.astype(f32)[order].astype(np.float64)

    live = (va_s > 0) & (hs_s >= 0) & np.isfinite(hs_s) \
        & np.isfinite(sp_s).all(axis=1)
    sx = np.clip(sp_s[:, 0], -1e15, 1e15)
    sy = np.clip(sp_s[:, 1], -1e15, 1e15)
    hs2 = np.minimum(hs_s * hs_s, 1e12)

    # per (core, tile) voxel index lists (order preserved = back-to-front)
    lists = []
    vmax = 1
    m = 0.1
    for k in range(NCORES):
        row0 = k * BAND
        core_lists = []
        for tr in range(TR):
            for tcc in range(TC):
                y0, x0 = row0 + tr * R_, tcc * C_
                sel = (live
                       & (sx + hs_s + m >= x0) & (sx - hs_s - m <= x0 + C_ - 1)
                       & (sy + hs_s + m >= y0) & (sy - hs_s - m <= y0 + R_ - 1))
                idx = np.nonzero(sel)[0]
                core_lists.append(idx)
                vmax = max(vmax, len(idx))
        lists.append(core_lists)

    if vmax <= 128:
        V = max(16, (vmax + 15) // 16 * 16)
    else:
        V = (vmax + P - 1) // P * P
    VC = (V + P - 1) // P

    # shared pixel-feature table (tile-centered coords -> same for all tiles)
    pxx = np.tile(np.arange(C_), R_) - (C_ - 1) / 2.0       # p = r*C_+c
    pyy = np.repeat(np.arange(R_), C_) - (R_ - 1) / 2.0
    pxf = np.stack([pxx, pyy, pxx * pxx + pyy * pyy,
                    np.ones(P)]).astype(np.float32)

    in_maps = []
    for k in range(NCORES):
        rhs8 = np.zeros((T, 4, 2 * V), np.float32)
        vab = np.zeros((T, V), np.float32)
        vals = np.zeros((T, V, 4), np.float32)
        row0 = k * BAND
        for t in range(T):
            tr, tcc = divmod(t, TC)
            y0, x0 = row0 + tr * R_, tcc * C_
            cx0, cy0 = x0 + (C_ - 1) / 2.0, y0 + (R_ - 1) / 2.0
            idx = lists[k][t]
            nv = len(idx)
            sxc, syc = sx[idx] - cx0, sy[idx] - cy0
            s2 = sxc * sxc + syc * syc
            rhs8[t, 0, 0:nv] = -2 * sxc
            rhs8[t, 1, 0:nv] = -2 * syc
            rhs8[t, 2, 0:nv] = 1.0
            rhs8[t, 3, 0:nv] = s2
            rhs8[t, 0, V:V + nv] = -2 * sxc
            rhs8[t, 1, V:V + nv] = -2 * syc
            rhs8[t, 2, V:V + nv] = 1.0
            rhs8[t, 3, V:V + nv] = s2 - hs2[idx]
            # padding voxels: rhs rows already 0 -> d2=0, t=0 -> masked only
            # by va=0; set pad mask row so t>0 kills them regardless
            if nv < V:
                rhs8[t, 2, nv:V] = 1.0
                rhs8[t, 3, nv:V] = 1e6          # d2 = 1e6 -> exp(-500) = 0
                rhs8[t, 2, V + nv:2 * V] = 1.0
                rhs8[t, 3, V + nv:2 * V] = 1e6  # t = 1e6 -> pen huge
            vab[t, 0:nv] = va_s[idx]
            vals[t, 0:nv, 0:3] = c_s[idx]
            vals[t, 0:nv, 3] = np.clip(d_s[idx], -1e30, 1e30) - FAR
        if V > P:
            # chunk-major storage [(c vp), f] so the device sees [vp, c, f]
            vals_pad = np.zeros((T, VC * P, 4), np.float32)
            vals_pad[:, 0:V, :] = vals
            vals_store = vals_pad
        else:
            vals_store = vals
        in_maps.append({"pxf": pxf, "rhs8": rhs8, "vab": vab,
                        "vals": vals_store,
                        "ident": np.eye(P, dtype=np.float32)})
    return V, in_maps


class _Runner:
    """Cached jitted shard_map executor for a built Bass module (the stock
    run_bass_kernel_spmd re-traces jax every call, ~240ms)."""

    def __init__(self, nc, n_cores):
        import jax
        import numpy as _np
        from jax.sharding import Mesh, PartitionSpec
        from jax.experimental.shard_map import shard_map
        import concourse.mybir as mybir
        from concourse import bass2jax

        bass2jax.install_neuronx_cc_hook()
        self.n_cores = n_cores
        in_names, out_names, out_avals, zero_outs = [], [], [], []
        for alloc in nc.m.functions[0].allocations:
            if not isinstance(alloc, mybir.MemoryLocationSet):
                continue
            name = alloc.memorylocations[0].name
            if alloc.kind == "ExternalInput":
                if (nc.partition_id_tensor is None
                        or name != nc.partition_id_tensor.name):
                    in_names.append(name)
            elif alloc.kind == "ExternalOutput":
                out_names.append(name)
                shape = tuple(alloc.tensor_shape)
                dtype = mybir.dt.np(alloc.dtype)
                out_avals.append(jax.core.ShapedArray(shape, dtype))
                zero_outs.append(_np.zeros(shape, dtype))
        self.in_names, self.out_names = list(in_names), list(out_names)
        self.zero_outs = zero_outs
        n_params, n_outs = len(in_names), len(out_names)
        all_names = in_names + out_names
        partition_name = (nc.partition_id_tensor.name
                          if nc.partition_id_tensor else None)
        if partition_name is not None:
            all_names = all_names + [partition_name]

        def _body(*args):
            operands = list(args)
            if partition_name is not None:
                operands.append(bass2jax.partition_id_tensor())
            outs = bass2jax._bass_exec_p.bind(
                *operands, out_avals=tuple(out_avals),
                in_names=tuple(all_names),
                out_names=tuple(out_names), lowering_input_output_aliases=(),
                sim_require_finite=True, sim_require_nnan=True, nc=nc)
            return tuple(outs)

        devices = jax.devices()[:n_cores]
        mesh = Mesh(np.asarray(devices), ("core",))
        self._fn = jax.jit(
            shard_map(_body, mesh=mesh,
                      in_specs=(PartitionSpec("core"),) * (n_params + n_outs),
                      out_specs=(PartitionSpec("core"),) * n_outs,
                      check_rep=False),
            donate_argnums=tuple(range(n_params, n_params + n_outs)),
            keep_unused=True)

    def __call__(self, in_maps):
        import jax
        concat_in = [np.concatenate([np.asarray(m[n]) for m in in_maps],
                                    axis=0) for n in self.in_names]
        concat_zero = [np.concatenate([z] * self.n_cores, axis=0)
                       for z in self.zero_outs]
        outs = jax.block_until_ready(self._fn(*concat_in, *concat_zero))
        results = []
        for c in range(self.n_cores):
            d = {}
            for i, n in enumerate(self.out_names):
                per = self.zero_outs[i].shape[0]
                d[n] = np.asarray(outs[i][c * per:(c + 1) * per])
            results.append(d)
        return results


def _get_runner(V):
    if V not in _nc_cache:
        nc = _build(V)
        _nc_cache[V] = _Runner(nc, NCORES)
    return _nc_cache[V]


def kernel(positions, sizes, densities, colors, camera_matrix, intrinsics):
    V, in_maps = _prep(positions, sizes, densities, colors,
                       camera_matrix, intrinsics)
    runner = _get_runner(V)

    class _R:
        results = runner(in_maps)
    res = _R()
    rgb = np.zeros((H, W_IMG, 3), np.float32)
    dep = np.zeros((H, W_IMG), np.float32)
    alp = np.zeros((H, W_IMG), np.float32)
    # unscatter: pixel p = r*C_+c of tile t = tr*TC+tc -> (k*BAND+tr*R_+r,
    # tc*C_+c)
    for k in range(NCORES):
        r = res.results[k]
        band_rgb = (r["rgb"].reshape(R_, C_, TR, TC, 3)
                    .transpose(2, 0, 3, 1, 4).reshape(BAND, W_IMG, 3))
        band_dep = (r["dep"].reshape(R_, C_, TR, TC)
                    .transpose(2, 0, 3, 1).reshape(BAND, W_IMG))
        band_alp = (r["alp"].reshape(R_, C_, TR, TC)
                    .transpose(2, 0, 3, 1).reshape(BAND, W_IMG))
        rgb[k * BAND:(k + 1) * BAND] = band_rgb
        dep[k * BAND:(k + 1) * BAND] = band_dep
        alp[k * BAND:(k + 1) * BAND] = band_alp
    return rgb, dep, alp


# revision 19
# speedup vs baseline: 6758.3184x; 6758.3184x over previous
"""Differentiable voxel rasterizer — Trainium2 Bass kernel (8 NeuronCores).

Contract: kernel(**inputs) takes FULL inputs (positions [512,3], sizes [512],
densities [512], colors [512,3], camera_matrix [4,4], intrinsics [3,3]) and
returns the FULL output tuple (rgb [256,256,3], depth [256,256],
alpha [256,256]) matching reference.reference().

Strategy (sharding = image plane): each of 8 cores renders a 32-row band.
Within a band, 64 tiles of 8x16 px (partition dim = 128 pixels). Host
projects voxels, culls per tile (exact: outside the circular footprint the
reference weight is identically 0), sorts back-to-front, and packs per-tile
tables. Device computes, per tile:
    d2|t       via one PE matmul (rank-4 expansion, tile-centered coords)
    dist       = sqrt(relu(d2))            (ACT)
    pen        = relu(BIG*t)               (DVE, t = d2-hs^2 mask penalty)
    E          = exp(-0.5*(dist+pen))      (ACT; gpsimd add)
    W          = E*va                      (gpsimd)
    P          = prefix prod of (1-W)      (DVE tensor_tensor_scan)
    blend_v    = P_{v-1}-P_v (telescoping) -> omb = 1-blend via one stt
    R          = suffix prod of omb        (DVE reversed scan)
    BQ_v       = R_{v+1}-R_v (telescoping) = blend_v * prod_{u>v} omb_u
    rgb/depth  = matmul(BQ^T, [c | d-FAR]) (PE transpose + PE matmul)
    alpha      = 1 - P_total
Everything is NaN-free by construction (device NaNs crash the NRT).
"""
import os
import sys

for _p in ("/opt/trn_rl_repo", os.path.expanduser("~/.axon_site/_ro/trn_rl_repo")):
    if os.path.isdir(_p) and _p not in sys.path:
        sys.path.insert(0, _p)

import numpy as np

H, W_IMG = 256, 256
NEAR, FAR = 0.1, 100.0
NVOX = 512
NCORES = 8
BAND = H // NCORES          # 32 rows per core
TR, TC = 4, 16              # tile grid per band
R_, C_ = 8, 16              # tile pixel shape (R_*C_ = 128 partitions)
T = TR * TC                 # 64 tiles per core
P = 128
BIG = 1e8

_nc_cache: dict = {}


def _build(V, rep=1):
    """Build + compile the per-core module for V voxels per tile.

    rep > 1 wraps the whole tile pass in a For_i loop that recomputes the
    identical result rep times — used only for HW-time measurement (the
    axon dispatch overhead is ~0.3-0.5 s, so single-pass wall time says
    nothing about kernel time)."""
    import concourse.bacc as bacc
    import concourse.bass as bass
    import concourse.mybir as mybir
    from concourse.tile import TileContext

    F32 = mybir.dt.float32
    Alu = mybir.AluOpType
    Act = mybir.ActivationFunctionType
    VC = (V + P - 1) // P          # voxel chunks for transpose/reduction
    assert V <= 512

    nc = bacc.Bacc("TRN2", target_bir_lowering=False, debug=False)
    # pxf is shared by all tiles: coords are tile-centered, so the pixel
    # feature table is identical for every tile/core.
    pxf_d = nc.dram_tensor("pxf", [4, P], F32, kind="ExternalInput")
    rhs_d = nc.dram_tensor("rhs8", [T, 4, 2 * V], F32, kind="ExternalInput")
    vab_d = nc.dram_tensor("vab", [T, V], F32, kind="ExternalInput")
    vals_d = nc.dram_tensor("vals", [T, VC * P if V > P else V, 4], F32,
                            kind="ExternalInput")
    id_d = nc.dram_tensor("ident", [P, P], F32, kind="ExternalInput")
    # outputs in device-natural layout [pixel-in-tile, tile(,ch)];
    # host unscatters to image layout during gather
    rgb_d = nc.dram_tensor("rgb", [P, T, 3], F32, kind="ExternalOutput")
    dep_d = nc.dram_tensor("dep", [P, T], F32, kind="ExternalOutput")
    alp_d = nc.dram_tensor("alp", [P, T], F32, kind="ExternalOutput")

    with TileContext(nc) as tc:
        with tc.tile_pool(name="const", bufs=1) as cpool, \
             tc.tile_pool(name="work", bufs=3) as pool, \
             tc.tile_pool(name="pwork", bufs=2, space="PSUM") as ppool, \
             tc.tile_pool(name="pout", bufs=1, space="PSUM") as opool:
            ident = cpool.tile([P, P], F32)
            zeros = cpool.tile([P, V], F32)
            astage = cpool.tile([P, T], F32)
            pxf = cpool.tile([4, P], F32)
            nc.sync.dma_start(out=ident[:], in_=id_d.ap())
            nc.sync.dma_start(out=pxf[:], in_=pxf_d.ap())
            nc.vector.memset(zeros[:], 0)
            outp = opool.tile([P, 4 * T], F32)

            def tile_pass():
                for t in range(T):
                    one_tile(t)

            def one_tile(t):
                rhs8 = pool.tile([4, 2 * V], F32)
                vab = pool.tile([P, V], F32)
                vals = pool.tile([P, VC, 4], F32)
                nc.sync.dma_start(out=rhs8[:], in_=rhs_d.ap()[t])
                nc.sync.dma_start(
                    out=vals[0:min(P, V), :, :],
                    in_=vals_d.ap()[t].rearrange("(c vp) f -> vp c f", c=VC))
                row = vab_d.ap()[t:t + 1, :]
                nc.sync.dma_start(
                    out=vab[:],
                    in_=bass.AP(row.tensor, row.offset, [[0, P], [1, V]]))

                d2t = ppool.tile([P, 2 * V], F32)
                nc.tensor.matmul(d2t[:], lhsT=pxf[:], rhs=rhs8[:],
                                 start=True, stop=True)

                d2c = pool.tile([P, V], F32)
                dist = pool.tile([P, V], F32)
                pen = pool.tile([P, V], F32)
                de = pool.tile([P, V], F32)
                E = pool.tile([P, V], F32)
                Wt = pool.tile([P, V], F32)
                omw = pool.tile([P, V], F32)
                Pbuf = pool.tile([P, V + 1], F32)
                omb = pool.tile([P, V], F32)
                Qbuf = pool.tile([P, V + 1], F32)
                BQ = pool.tile([P, V], F32)

                nc.scalar.activation(out=d2c[:], in_=d2t[:, 0:V], func=Act.Relu)
                nc.scalar.activation(out=dist[:], in_=d2c[:], func=Act.Sqrt)
                nc.vector.tensor_scalar(out=pen[:], in0=d2t[:, V:2 * V],
                                        scalar1=BIG, scalar2=0.0,
                                        op0=Alu.mult, op1=Alu.max)
                nc.gpsimd.tensor_tensor(out=de[:], in0=dist[:], in1=pen[:],
                                        op=Alu.add)
                nc.scalar.activation(out=E[:], in_=de[:], func=Act.Exp,
                                     scale=-0.5)
                nc.gpsimd.tensor_tensor(out=Wt[:], in0=E[:], in1=vab[:],
                                        op=Alu.mult)
                nc.vector.tensor_scalar(out=omw[:], in0=Wt[:], scalar1=-1.0,
                                        scalar2=1.0, op0=Alu.mult, op1=Alu.add)
                nc.gpsimd.memset(Pbuf[:, 0:1], 1.0)
                nc.gpsimd.memset(Qbuf[:, V:V + 1], 1.0)
                nc.vector.tensor_tensor_scan(
                    out=Pbuf[:, 1:V + 1], data0=omw[:], data1=zeros[:],
                    initial=1.0, op0=Alu.mult, op1=Alu.max)
                nc.vector.scalar_tensor_tensor(
                    out=omb[:], in0=Pbuf[:, 1:V + 1], scalar=1.0,
                    in1=Pbuf[:, 0:V], op0=Alu.add, op1=Alu.subtract)
                nc.vector.tensor_tensor_scan(
                    out=Qbuf[:, 0:V][:, ::-1], data0=omb[:][:, ::-1],
                    data1=zeros[:], initial=1.0, op0=Alu.mult, op1=Alu.max)
                nc.vector.tensor_tensor(out=BQ[:], in0=Qbuf[:, 1:V + 1],
                                        in1=Qbuf[:, 0:V], op=Alu.subtract)
                nc.gpsimd.tensor_copy(out=astage[:, t:t + 1],
                                      in_=Pbuf[:, V:V + 1])

                bqT = ppool.tile([P, VC * P], F32)
                bqTs = pool.tile([P, VC * P], F32)
                for ch in range(VC):
                    cl = min(P, V - ch * P)
                    nc.tensor.transpose(bqT[0:cl, ch * P:ch * P + P],
                                        BQ[:, ch * P:ch * P + cl], ident[:])
                    nc.scalar.copy(out=bqTs[0:cl, ch * P:ch * P + P],
                                   in_=bqT[0:cl, ch * P:ch * P + P])
                for ch in range(VC):
                    cl = min(P, V - ch * P)
                    nc.tensor.matmul(outp[:, 4 * t:4 * t + 4],
                                     lhsT=bqTs[0:cl, ch * P:ch * P + P],
                                     rhs=vals[0:cl, ch, :],
                                     start=(ch == 0), stop=(ch == VC - 1))

            if rep == 1:
                tile_pass()
            else:
                with tc.For_i(0, rep, 1) as _:
                    tile_pass()

            rgbsb = cpool.tile([P, T, 3], F32)
            depsb = cpool.tile([P, T], F32)
            alpsb = cpool.tile([P, T], F32)
            outv = outp[:].rearrange("p (t f) -> p t f", f=4)
            nc.vector.tensor_copy(out=rgbsb[:], in_=outv[:, :, 0:3])
            nc.vector.tensor_scalar(out=depsb[:], in0=outv[:, :, 3:4],
                                    scalar1=1.0, scalar2=FAR,
                                    op0=Alu.mult, op1=Alu.add)
            nc.vector.tensor_scalar(out=alpsb[:], in0=astage[:], scalar1=-1.0,
                                    scalar2=1.0, op0=Alu.mult, op1=Alu.add)
            nc.sync.dma_start(out=rgb_d.ap(), in_=rgbsb[:])
            nc.sync.dma_start(out=dep_d.ap(), in_=depsb[:])
            nc.sync.dma_start(out=alp_d.ap(), in_=alpsb[:])
    nc.compile()
    return nc


def _softplus(x):
    return np.logaddexp(0.0, x)


def _prep(positions, sizes, densities, colors, camera_matrix, intrinsics):
    """Mirror the reference projection in numpy fp32; build per-tile tables."""
    f32 = np.float32
    pos = positions.astype(f32)
    n = pos.shape[0]
    hom = np.concatenate([pos, np.ones((n, 1), f32)], axis=1)
    cam = hom @ camera_matrix.astype(f32).T
    with np.errstate(divide="ignore", invalid="ignore", over="ignore"):
        cam3 = cam[:, :3] / cam[:, 3:4]
        scr = cam3 @ intrinsics.astype(f32).T
        sp = scr[:, :2] / scr[:, 2:3]
        depths = cam3[:, 2]
        fx = intrinsics.astype(f32)[0, 0]
        ssize = sizes.astype(f32) * fx / np.maximum(depths, f32(0.1))
        x, y = sp[:, 0], sp[:, 1]
        vis = ((depths > NEAR) & (depths < FAR)
               & (x + ssize >= 0) & (x - ssize < W_IMG)
               & (y + ssize >= 0) & (y - ssize < H))
        vis = vis & np.isfinite(x) & np.isfinite(y) & np.isfinite(ssize)
        valpha = np.clip(
            1.0 - np.exp(-_softplus(densities.astype(f32)) * sizes.astype(f32)),
            0.0, 1.0) * vis.astype(f32)
    order = np.argsort(-np.where(np.isnan(depths), -np.inf, depths),
                       kind="stable")
    sp_s = sp[order].astype(np.float64)
    hs_s = 0.5 * ssize[order].astype(np.float64)
    va_s = valpha[order].astype(np.float64)
    d_s = depths[order].astype(np.float64)
    c_s = colors.astype(f32)[order].astype(np.float64)

    live = (va_s > 0) & (hs_s >= 0) & np.isfinite(hs_s) \
        & np.isfinite(sp_s).all(axis=1)
    sx = np.clip(sp_s[:, 0], -1e15, 1e15)
    sy = np.clip(sp_s[:, 1], -1e15, 1e15)
    hs2 = np.minimum(hs_s * hs_s, 1e12)

    # per (core, tile) voxel index lists (order preserved = back-to-front)
    lists = []
    vmax = 1
    m = 0.1
    for k in range(NCORES):
        row0 = k * BAND
        core_lists = []
        for tr in range(TR):
            for tcc in range(TC):
                y0, x0 = row0 + tr * R_, tcc * C_
                sel = (live
                       & (sx + hs_s + m >= x0) & (sx - hs_s - m <= x0 + C_ - 1)
                       & (sy + hs_s + m >= y0) & (sy - hs_s - m <= y0 + R_ - 1))
                idx = np.nonzero(sel)[0]
                core_lists.append(idx)
                vmax = max(vmax, len(idx))
        lists.append(core_lists)

    if vmax <= 128:
        V = max(16, (vmax + 15) // 16 * 16)
    else:
        V = (vmax + P - 1) // P * P
    VC = (V + P - 1) // P

    # shared pixel-feature table (tile-centered coords -> same for all tiles)
    pxx = np.tile(np.arange(C_), R_) - (C_ - 1) / 2.0       # p = r*C_+c
    pyy = np.repeat(np.arange(R_), C_) - (R_ - 1) / 2.0
    pxf = np.stack([pxx, pyy, pxx * pxx + pyy * pyy,
                    np.ones(P)]).astype(np.float32)

    in_maps = []
    for k in range(NCORES):
        rhs8 = np.zeros((T, 4, 2 * V), np.float32)
        vab = np.zeros((T, V), np.float32)
        vals = np.zeros((T, V, 4), np.float32)
        row0 = k * BAND
        for t in range(T):
            tr, tcc = divmod(t, TC)
            y0, x0 = row0 + tr * R_, tcc * C_
            cx0, cy0 = x0 + (C_ - 1) / 2.0, y0 + (R_ - 1) / 2.0
            idx = lists[k][t]
            nv = len(idx)
            sxc, syc = sx[idx] - cx0, sy[idx] - cy0
            s2 = sxc * sxc + syc * syc
            rhs8[t, 0, 0:nv] = -2 * sxc
            rhs8[t, 1, 0:nv] = -2 * syc
            rhs8[t, 2, 0:nv] = 1.0
            rhs8[t, 3, 0:nv] = s2
            rhs8[t, 0, V:V + nv] = -2 * sxc
            rhs8[t, 1, V:V + nv] = -2 * syc
            rhs8[t, 2, V:V + nv] = 1.0
            rhs8[t, 3, V:V + nv] = s2 - hs2[idx]
            # padding voxels: rhs rows already 0 -> d2=0, t=0 -> masked only
            # by va=0; set pad mask row so t>0 kills them regardless
            if nv < V:
                rhs8[t, 2, nv:V] = 1.0
                rhs8[t, 3, nv:V] = 1e6          # d2 = 1e6 -> exp(-500) = 0
                rhs8[t, 2, V + nv:2 * V] = 1.0
                rhs8[t, 3, V + nv:2 * V] = 1e6  # t = 1e6 -> pen huge
            vab[t, 0:nv] = va_s[idx]
            vals[t, 0:nv, 0:3] = c_s[idx]
            vals[t, 0:nv, 3] = np.clip(d_s[idx], -1e30, 1e30) - FAR
        if V > P:
            # chunk-major storage [(c vp), f] so the device sees [vp, c, f]
            vals_pad = np.zeros((T, VC * P, 4), np.float32)
            vals_pad[:, 0:V, :] = vals
            vals_store = vals_pad
        else:
            vals_store = vals
        in_maps.append({"pxf": pxf, "rhs8": rhs8, "vab": vab,
                        "vals": vals_store,
                        "ident": np.eye(P, dtype=np.float32)})
    return V, in_maps


def kernel(positions, sizes, densities, colors, camera_matrix, intrinsics):
    from concourse import bass_utils

    V, in_maps = _prep(positions, sizes, densities, colors,
                       camera_matrix, intrinsics)
    if V not in _nc_cache:
        _nc_cache[V] = _build(V)
    res = bass_utils.run_bass_kernel_spmd(_nc_cache[V], in_maps,
                                          core_ids=list(range(NCORES)))
    rgb = np.zeros((H, W_IMG, 3), np.float32)
    dep = np.zeros((H, W_IMG), np.float32)
    alp = np.zeros((H, W_IMG), np.float32)
    # unscatter: pixel p = r*C_+c of tile t = tr*TC+tc -> (k*BAND+tr*R_+r,
    # tc*C_+c)
    for k in range(NCORES):
        r = res.results[k]
        band_rgb = (r["rgb"].reshape(R_, C_, TR, TC, 3)
                    .transpose(2, 0, 3, 1, 4).reshape(BAND, W_IMG, 3))
        band_dep = (r["dep"].reshape(R_, C_, TR, TC)
                    .transpose(2, 0, 3, 1).reshape(BAND, W_IMG))
        band_alp = (r["alp"].reshape(R_, C_, TR, TC)
                    .transpose(2, 0, 3, 1).reshape(BAND, W_IMG))
        rgb[k * BAND:(k + 1) * BAND] = band_rgb
        dep[k * BAND:(k + 1) * BAND] = band_dep
        alp[k * BAND:(k + 1) * BAND] = band_alp
    return rgb, dep, alp
